# revision 3
# baseline (speedup 1.0000x reference)
"""Trainium2 Bass kernel for nn_ContentEncoder (4-ary tree GNN message passing).

Strategy (8 NeuronCores):
- Each core owns 2 of the 16 depth-2 subtrees (10922 nodes, contiguous per level).
- Everything on-device is D-major: [d_model on partitions (4 chunks of 128), nodes on free].
  Host pre-transposes text/img/bgimg and pre-gathers E_order[order]/E_tag[tag] into one
  [2816, NSLAB] bf16 slab per core -> zero on-device transposes/gathers.
- Embed: 14 accumulating matmuls per d-chunk (text 6 + img 4 + bg 4), elementwise-max
  folded with the two gathered tables via DVE.
- Up sweep L7->L2 and down sweep L3->L8 as fp32r MLPs (full PE rate at N>=256).
  h_leaf is constant-folded into biases (up-L7 and down-L8 lose half their MLP1 K).
- Tiny top of tree (21 nodes): 32KB AllGather of level-2 u vectors, then redundant
  compute on every core; host only assembles rows 0..4.
"""
import sys
sys.path.insert(0, '/opt/trn_rl_repo')

import numpy as np
import ml_dtypes

import concourse.bass as bass
import concourse.mybir as mybir
import concourse.tile as tile
from concourse import bacc
from concourse.bass_utils import run_bass_kernel_spmd

F32 = mybir.dt.float32
F32R = mybir.dt.float32r
BF16 = mybir.dt.bfloat16

USE_BF16 = True          # embed inputs/weights in bf16 (halves input DMA)
NCORES = 8
P = 128
D = 512
NLOW = 170               # slab cols for levels 2..5 (resident h)
L6_OFF = 170             # level 6 at cols 170..681
L7_OFF = 682             # level 7 at cols 682..2729
L8_OFF = 2816            # level 8 starts here (64B-aligned-ish padding gap)
NSLAB = 2816 + 8192      # 11008
NSCR = 512 + 2048        # scratch cols: L6 at 0..511, L7 at 512..2559

# per-core level geometry: level l in 2..8
M_L = {l: 2 * 4 ** (l - 2) for l in range(2, 9)}          # cols per level
SLAB_OFF = {2: 0, 3: 2, 4: 10, 5: 42, 6: 170, 7: 682, 8: 2816}
GOFF = {l: (4 ** l - 1) // 3 for l in range(0, 9)}         # global level offsets

XDT = BF16 if USE_BF16 else F32R
XNP = ml_dtypes.bfloat16 if USE_BF16 else np.float32

RG = [list(range(NCORES))]

_cache = {}


def build_program():
    if 'nc' in _cache:
        return _cache['nc']
    nc = bacc.Bacc("TRN2", target_bir_lowering=False, debug=False, num_devices=NCORES)

    xT = nc.dram_tensor("xT", [2816, NSLAB], XDT, kind="ExternalInput")
    wemb = nc.dram_tensor("wemb", [1792, 512], XDT, kind="ExternalInput")
    wmlp = nc.dram_tensor("wmlp", [1536, 512], F32R, kind="ExternalInput")
    biases = nc.dram_tensor("biases", [8, 512], F32, kind="ExternalInput")
    hl1T = nc.dram_tensor("hl1T", [512, 4], F32R, kind="ExternalInput")
    hrootT = nc.dram_tensor("hrootT", [512, 1], F32R, kind="ExternalInput")
    selmask = nc.dram_tensor("selmask", [128, 8], F32, kind="ExternalInput")

    outT = nc.dram_tensor("outT", [512, NSLAB], F32, kind="ExternalOutput")
    d1out = nc.dram_tensor("d1out", [4, 512], F32R, kind="ExternalOutput")

    Relu = mybir.ActivationFunctionType.Relu
    Ident = mybir.ActivationFunctionType.Identity
    AX = mybir.AxisListType.X
    ADD = mybir.AluOpType.add
    MAX = mybir.AluOpType.max

    xlo_v = xT.ap().rearrange("(c p) n -> p c n", p=P)[:, 0:11, :]
    xhi_v = xT.ap().rearrange("(c p) n -> p c n", p=P)[:, 11:22, :]
    outT_v = outT.ap().rearrange("(c p) n -> p c n", p=P)

    with tile.TileContext(nc) as tc:
        with (
            tc.tile_pool(name="const", bufs=1) as cst,
            tc.tile_pool(name="work", bufs=1) as wk,
            tc.tile_pool(name="psp", bufs=1, space="PSUM") as psp,
            tc.tile_pool(name="dram", bufs=1, space="DRAM") as dram,
        ):
            # ---------- constants ----------
            wemb_sb = cst.tile([P, 14, 512], XDT, tag="wemb")
            nc.sync.dma_start(wemb_sb[:], wemb.ap().rearrange("(c p) m -> p c m", p=P))
            wmlp_sb = cst.tile([P, 12, 512], F32R, tag="wmlp")
            nc.sync.dma_start(wmlp_sb[:], wmlp.ap().rearrange("(c p) m -> p c m", p=P))
            bcol = cst.tile([P, 8, 4], F32, tag="bcol")
            nc.sync.dma_start(bcol[:], biases.ap().rearrange("r (m p) -> p r m", p=P))
            hl1_sb = cst.tile([P, 4, 4], F32R, tag="hl1")
            nc.sync.dma_start(hl1_sb[:], hl1T.ap().rearrange("(c p) n -> p c n", p=P))
            hroot_sb = cst.tile([P, 4, 1], F32R, tag="hroot")
            nc.sync.dma_start(hroot_sb[:], hrootT.ap().rearrange("(c p) n -> p c n", p=P))
            selm_sb = cst.tile([P, 8], F32, tag="selm")
            nc.sync.dma_start(selm_sb[:], selmask.ap())

            hlow = cst.tile([P, 4, NLOW], F32R, tag="hlow")

            hscr = dram.tile([4, P, NSCR], F32R, tag="hscr")
            uscr = dram.tile([4, P, NSCR], F32R, tag="uscr")
            u2m_t = dram.tile([2, 512], F32R, tag="u2m")
            u2all_t = dram.tile([16, 512], F32R, tag="u2all")

            def bias_ap(r, m):
                return bcol[:, r, m:m + 1]

            def ms(m):
                return slice(m * P, (m + 1) * P)

            # ---------- embed ----------
            def embed_block(col0, w, h_out):
                lo = wk.tile([P, 11, w], XDT, tag="slab", bufs=2)
                hi = wk.tile([P, 11, w], XDT, tag="slab", bufs=2)
                nc.sync.dma_start(lo[:], xlo_v[:, :, col0:col0 + w])
                nc.sync.dma_start(hi[:], xhi_v[:, :, col0:col0 + w])
                for m in range(4):
                    ps_t = psp.tile([P, w], F32, tag="pse", bufs=4)
                    for j, ch in enumerate(range(0, 6)):
                        nc.tensor.matmul(ps_t[:], wemb_sb[:, ch, ms(m)], lo[:, ch, :],
                                         start=(j == 0), stop=(j == 5))
                    t0 = wk.tile([P, w], F32R, tag="embtmp", bufs=3)
                    nc.scalar.activation(t0[:], ps_t[:], Ident, bias=bias_ap(0, m))
                    ps_i = psp.tile([P, w], F32, tag="pse", bufs=4)
                    for j, ch in enumerate(range(6, 10)):
                        nc.tensor.matmul(ps_i[:], wemb_sb[:, ch, ms(m)], lo[:, ch, :],
                                         start=(j == 0), stop=(j == 3))
                    t1 = wk.tile([P, w], F32R, tag="embtmp", bufs=3)
                    nc.vector.scalar_tensor_tensor(
                        out=t1[:], in0=ps_i[:], scalar=bias_ap(1, m), in1=t0[:],
                        op0=ADD, op1=MAX)
                    ps_b = psp.tile([P, w], F32, tag="pse", bufs=4)
                    for j, ch in enumerate(range(10, 14)):
                        src = lo[:, ch, :] if ch < 11 else hi[:, ch - 11, :]
                        nc.tensor.matmul(ps_b[:], wemb_sb[:, ch, ms(m)], src,
                                         start=(j == 0), stop=(j == 3))
                    t2 = wk.tile([P, w], F32R, tag="embtmp", bufs=3)
                    nc.vector.scalar_tensor_tensor(
                        out=t2[:], in0=ps_b[:], scalar=bias_ap(2, m), in1=t1[:],
                        op0=ADD, op1=MAX)
                    t3 = wk.tile([P, w], F32R, tag="embtmp", bufs=3)
                    nc.vector.tensor_max(t3[:], t2[:], hi[:, 3 + m, :])
                    nc.vector.tensor_max(h_out(m), t3[:], hi[:, 7 + m, :])

            # ---------- generic 2-layer MLP (4 d-chunks) ----------
            def mlp_block(w, nlo, get_lo, nhi, get_hi, b1row, out_ap=None, finish=None):
                y1 = wk.tile([P, 4, w], F32R, tag="y1", bufs=2)
                total = nlo + nhi
                for m in range(4):
                    ps1 = psp.tile([P, w], F32, tag="psm1", bufs=2)
                    i = 0
                    for k in range(nlo):
                        nc.tensor.matmul(ps1[:], wmlp_sb[:, k, ms(m)], get_lo(k),
                                         start=(i == 0), stop=(i == total - 1))
                        i += 1
                    for k in range(nhi):
                        nc.tensor.matmul(ps1[:], wmlp_sb[:, 4 + k, ms(m)], get_hi(k),
                                         start=(i == 0), stop=(i == total - 1))
                        i += 1
                    nc.scalar.activation(y1[:, m, :], ps1[:], Relu, bias=bias_ap(b1row, m))
                for m in range(4):
                    ps2 = psp.tile([P, w], F32, tag="psm2", bufs=2)
                    for k in range(4):
                        nc.tensor.matmul(ps2[:], wmlp_sb[:, 8 + k, ms(m)], y1[:, k, :],
                                         start=(k == 0), stop=(k == 3))
                    if finish is not None:
                        finish(m, ps2)
                    else:
                        nc.scalar.activation(out_ap(m), ps2[:], Ident, bias=bias_ap(4, m))

            # =========================================================
            # P1/P2: embed + upward sweep
            # =========================================================
            # low levels 2..5 -> resident hlow
            embed_block(0, NLOW, lambda m: hlow[:, m, :])

            # level 7: 4 tiles of 512, fused embed + up-MLP (cu = h_leaf folded: b1row=5)
            u7 = wk.tile([P, 4, 2048], F32R, tag="big2048", bufs=1)
            for t in range(4):
                hst = wk.tile([P, 4, 512], F32R, tag="hstream", bufs=2)
                embed_block(L7_OFF + 512 * t, 512, lambda m, _h=hst: _h[:, m, :])
                for c in range(4):
                    nc.scalar.dma_start(hscr[c][:, 512 + 512 * t:1024 + 512 * t],
                                        hst[:, c, :])
                mlp_block(512, 4, lambda k, _h=hst: _h[:, k, :], 0, None, b1row=5,
                          out_ap=lambda m, _t=t, _u=u7: _u[:, m, 512 * _t:512 * _t + 512])
            for c in range(4):
                nc.scalar.dma_start(uscr[c][:, 512:2560], u7[:, c, :])

            # level 6: 1 tile of 512 (streamed h)
            h6 = wk.tile([P, 4, 512], F32R, tag="hstream", bufs=2)
            embed_block(L6_OFF, 512, lambda m, _h=h6: _h[:, m, :])
            for c in range(4):
                nc.scalar.dma_start(hscr[c][:, 0:512], h6[:, c, :])
            cu6 = wk.tile([P, 4, 512], F32R, tag="cu", bufs=1)
            nc.vector.reduce_max(cu6[:], u7[:].rearrange("p c (n four) -> p c n four", four=4),
                                 axis=AX)
            u6 = wk.tile([P, 4, 512], F32R, tag="u6", bufs=1)
            mlp_block(512, 4, lambda k: h6[:, k, :], 4, lambda k: cu6[:, k, :], b1row=3,
                      out_ap=lambda m: u6[:, m, :])
            for c in range(4):
                nc.scalar.dma_start(uscr[c][:, 0:512], u6[:, c, :])

            # levels 5..2: resident h/u
            prev_u = {6: u6}
            for l in (5, 4, 3, 2):
                n = M_L[l]
                usrc = prev_u[l + 1]
                cu = wk.tile([P, 4, n], F32R, tag="cu", bufs=1)
                nc.vector.reduce_max(
                    cu[:], usrc[:].rearrange("p c (n four) -> p c n four", four=4), axis=AX)
                u_l = cst.tile([P, 4, n], F32R, tag=f"u{l}")
                h0 = SLAB_OFF[l]
                mlp_block(n, 4, lambda k, _h0=h0, _n=n: hlow[:, k, _h0:_h0 + _n],
                          4, lambda k, _cu=cu: _cu[:, k, :], b1row=3,
                          out_ap=lambda m, _u=u_l: _u[:, m, :])
                prev_u[l] = u_l

            # =========================================================
            # P3: top of tree (AllGather + redundant compute)
            # =========================================================
            u2_t = prev_u[2]
            for c in range(4):
                nc.sync.dma_start(u2m_t[:][:, c * P:(c + 1) * P].transpose([1, 0]),
                                  u2_t[:, c, :])
            nc.gpsimd.collective_compute(
                "AllGather", mybir.AluOpType.bypass, replica_groups=RG,
                ins=[u2m_t.opt()], outs=[u2all_t.opt()])
            u2dm = wk.tile([P, 4, 16], F32R, tag="top16", bufs=2)
            for c in range(4):
                nc.sync.dma_start(u2dm[:, c, :],
                                  u2all_t[:][:, c * P:(c + 1) * P].transpose([1, 0]))
            cu1 = wk.tile([P, 4, 4], F32R, tag="topsmall", bufs=4)
            nc.vector.reduce_max(
                cu1[:], u2dm[:].rearrange("p c (n four) -> p c n four", four=4), axis=AX)
            u1 = wk.tile([P, 4, 4], F32R, tag="topsmall", bufs=4)
            mlp_block(4, 4, lambda k: hl1_sb[:, k, :], 4, lambda k: cu1[:, k, :], b1row=3,
                      out_ap=lambda m: u1[:, m, :])
            d1 = wk.tile([P, 4, 4], F32R, tag="topsmall", bufs=4)
            mlp_block(4, 4, lambda k: u1[:, k, :],
                      4, lambda k: hroot_sb[:, k, :].unsqueeze(2).broadcast_to([P, 1, 4]),
                      b1row=3, out_ap=lambda m: d1[:, m, :])
            for c in range(4):
                nc.sync.dma_start(d1out.ap()[:, c * P:(c + 1) * P].transpose([1, 0]),
                                  d1[:, c, :])
            d2f = wk.tile([P, 4, 16], F32R, tag="top16", bufs=2)
            mlp_block(16, 4, lambda k: u2dm[:, k, :],
                      4, lambda k: d1[:, k, :].unsqueeze(2).broadcast_to([P, 4, 4]),
                      b1row=3, out_ap=lambda m: d2f[:, m, :])
            d2 = cst.tile([P, 4, 2], F32R, tag="d2")
            for c in range(4):
                tmp = wk.tile([P, 2, 8], F32, tag="toptmp", bufs=2)
                nc.vector.tensor_mul(
                    tmp[:], d2f[:, c, :].rearrange("p (g two) -> p two g", two=2),
                    selm_sb[:].unsqueeze(1).broadcast_to([P, 2, 8]))
                tmpr = wk.tile([P, 2], F32, tag="toptmp2", bufs=2)
                nc.vector.reduce_sum(tmpr[:], tmp[:], axis=AX)
                nc.vector.tensor_copy(d2[:, c, :], tmpr[:])
            o2 = wk.tile([P, 4, 2], F32, tag="topout", bufs=1)
            for m in range(4):
                nc.vector.tensor_add(o2[:, m, :], d2[:, m, :], hlow[:, m, 0:2])
            nc.scalar.dma_start(outT_v[:, :, 0:2], o2[:])

            # =========================================================
            # P4: downward sweep levels 3..7
            # =========================================================
            dres = {2: d2}
            for l in (3, 4, 5):
                n = M_L[l]
                u_l = prev_u[l]
                d_l = cst.tile([P, 4, n], F32R, tag=f"d{l}")
                mlp_block(n, 4, lambda k, _u=u_l: _u[:, k, :],
                          4, lambda k, _d=dres[l - 1], _n=n:
                              _d[:, k, :].unsqueeze(2).broadcast_to([P, _n // 4, 4]),
                          b1row=3, out_ap=lambda m, _d=d_l: _d[:, m, :])
                o = wk.tile([P, 4, n], F32, tag="ostage", bufs=2)
                h0 = SLAB_OFF[l]
                for m in range(4):
                    nc.vector.tensor_add(o[:, m, :], d_l[:, m, :], hlow[:, m, h0:h0 + n])
                nc.scalar.dma_start(outT_v[:, :, h0:h0 + n], o[:])
                dres[l] = d_l

            # level 6 (streamed u/h from scratch)
            u6s = wk.tile([P, 4, 512], F32R, tag="ustream", bufs=2)
            h6s = wk.tile([P, 4, 512], F32R, tag="hstream", bufs=2)
            for c in range(4):
                nc.sync.dma_start(u6s[:, c, :], uscr[c][:, 0:512])
                nc.sync.dma_start(h6s[:, c, :], hscr[c][:, 0:512])
            d6 = cst.tile([P, 4, 512], F32R, tag="d6")
            mlp_block(512, 4, lambda k: u6s[:, k, :],
                      4, lambda k: dres[5][:, k, :].unsqueeze(2).broadcast_to([P, 128, 4]),
                      b1row=3, out_ap=lambda m: d6[:, m, :])
            o6 = wk.tile([P, 4, 512], F32, tag="ostage", bufs=2)
            for m in range(4):
                nc.vector.tensor_add(o6[:, m, :], d6[:, m, :], h6s[:, m, :])
            nc.scalar.dma_start(outT_v[:, :, L6_OFF:L6_OFF + 512], o6[:])
            dres[6] = d6

            # level 7: 4 tiles (streamed u/h), d7 kept for level 8
            d7 = wk.tile([P, 4, 2048], F32R, tag="big2048", bufs=1)
            for t in range(4):
                ust = wk.tile([P, 4, 512], F32R, tag="ustream", bufs=2)
                hst = wk.tile([P, 4, 512], F32R, tag="hstream", bufs=2)
                for c in range(4):
                    nc.sync.dma_start(ust[:, c, :], uscr[c][:, 512 + 512 * t:1024 + 512 * t])
                    nc.sync.dma_start(hst[:, c, :], hscr[c][:, 512 + 512 * t:1024 + 512 * t])
                mlp_block(512, 4, lambda k, _u=ust: _u[:, k, :],
                          4, lambda k, _t=t: dres[6][:, k, 128 * _t:128 * _t + 128]
                              .unsqueeze(2).broadcast_to([P, 128, 4]),
                          b1row=3,
                          out_ap=lambda m, _t=t: d7[:, m, 512 * _t:512 * _t + 512])
                o = wk.tile([P, 4, 512], F32, tag="ostage", bufs=2)
                for m in range(4):
                    nc.vector.tensor_add(o[:, m, :], d7[:, m, 512 * t:512 * t + 512],
                                         hst[:, m, :])
                nc.scalar.dma_start(
                    outT_v[:, :, L7_OFF + 512 * t:L7_OFF + 512 * t + 512], o[:])

            # =========================================================
            # P5: level 8, fused embed + down-MLP + residual (16 tiles)
            # u_l8 == h_leaf folded into b1 (row 6); no d8 materialization.
            # =========================================================
            for t in range(16):
                hst = wk.tile([P, 4, 512], F32R, tag="hstream", bufs=2)
                embed_block(L8_OFF + 512 * t, 512, lambda m, _h=hst: _h[:, m, :])
                o = wk.tile([P, 4, 512], F32, tag="ostage", bufs=2)

                def fin(m, ps2, _o=o, _h=hst):
                    nc.vector.scalar_tensor_tensor(
                        out=_o[:, m, :], in0=ps2[:], scalar=bias_ap(4, m),
                        in1=_h[:, m, :], op0=ADD, op1=ADD)

                mlp_block(512, 0, None,
                          4, lambda k, _t=t: d7[:, k, 128 * _t:128 * _t + 128]
                              .unsqueeze(2).broadcast_to([P, 128, 4]),
                          b1row=6, finish=fin)
                nc.scalar.dma_start(
                    outT_v[:, :, L8_OFF + 512 * t:L8_OFF + 512 * t + 512], o[:])

    nc.compile()
    _cache['nc'] = nc
    return nc


def _core_rows(c, l):
    g0 = GOFF[l] + 2 * c * 4 ** (l - 2)
    return g0, g0 + M_L[l]


def prepare_inputs(inputs):
    inp = {k: np.asarray(v) for k, v in inputs.items()}
    order = np.asarray(inp['order'], np.int64)
    tag = np.asarray(inp['tag'], np.int64)
    N = order.shape[0]

    # host-side embed of top 5 nodes (f32, same math as reference)
    idx = np.arange(5)
    h_top = np.maximum.reduce([
        inp['E_order'][order[idx]], inp['E_tag'][tag[idx]],
        inp['text'][idx] @ inp['W_text'] + inp['b_text'],
        inp['img'][idx] @ inp['W_img'] + inp['b_img'],
        inp['bgimg'][idx] @ inp['W_bgimg'] + inp['b_bgimg'],
    ]).astype(np.float32)

    hl = inp['h_leaf'][0].astype(np.float32)
    W1 = inp['W1'].astype(np.float32)
    b1 = inp['b1'].astype(np.float32)
    biases = np.zeros((8, 512), np.float32)
    biases[0] = inp['b_text']
    biases[1] = inp['b_img']
    biases[2] = inp['b_bgimg']
    biases[3] = b1
    biases[4] = inp['b2']
    biases[5] = b1 + hl @ W1[512:]      # up level 7 (cu == h_leaf)
    biases[6] = b1 + hl @ W1[:512]      # down level 8 (u == h_leaf)

    wemb = np.concatenate([inp['W_text'], inp['W_img'], inp['W_bgimg']], 0).astype(XNP)
    wmlp = np.concatenate([W1[:512], W1[512:], inp['W2']], 0).astype(np.float32)
    hl1T = np.ascontiguousarray(h_top[1:5].T)
    hrootT = np.ascontiguousarray(inp['h_root'].astype(np.float32).T)

    # global D-major feature bank [2816, N]
    Xall = np.empty((2816, N), XNP)
    Xall[0:768] = inp['text'].T
    Xall[768:1280] = inp['img'].T
    Xall[1280:1792] = inp['bgimg'].T
    Xall[1792:2304] = inp['E_order'][order].T
    Xall[2304:2816] = inp['E_tag'][tag].T

    in_maps = []
    for c in range(NCORES):
        slab = np.zeros((2816, NSLAB), XNP)
        for l in range(2, 9):
            g0, g1 = _core_rows(c, l)
            o = SLAB_OFF[l]
            slab[:, o:o + M_L[l]] = Xall[:, g0:g1]
        sel = np.zeros((128, 8), np.float32)
        sel[:, c] = 1.0
        in_maps.append(dict(xT=slab, wemb=wemb, wmlp=wmlp, biases=biases,
                            hl1T=hl1T, hrootT=hrootT, selmask=sel))

    aux = dict(h_top=h_top, h_root=inp['h_root'].astype(np.float32), N=N)
    return in_maps, aux


def assemble(results, aux):
    N = aux['N']
    out = np.empty((N, 512), np.float32)
    out[0] = aux['h_root'][0] + aux['h_top'][0]
    out[1:5] = results[0]['d1out'] + aux['h_top'][1:5]
    for c in range(NCORES):
        oT = results[c]['outT']
        for l in range(2, 9):
            g0, g1 = _core_rows(c, l)
            o = SLAB_OFF[l]
            out[g0:g1] = oT[:, o:o + M_L[l]].T
    return out


def kernel(**inputs):
    nc = build_program()
    in_maps, aux = prepare_inputs(inputs)
    res = run_bass_kernel_spmd(nc, in_maps, list(range(NCORES)))
    return assemble(res.results, aux)
